# revision 96
# baseline (speedup 1.0000x reference)
"""AlphaPermutationLayer Trainium2 kernel.

out[i, j] = sum_k softmax(alpha/T)[k] * (perm[k, i] == j),  N=2048, K=64.

Sharding: output ROWS across the 8 cores (row i depends only on perm[:, i]
and alpha — no collective).  Per core (256 rows), digit-split
j = jq*64 + jf (jq in [0,32), jf in [0,64)); one matmul per row i:
    out_i[jq, jf] = sum_k A_i[k, jq] * B_i[k, jf]
with A = alpha-scaled onehot(perm>>6) stationary ([64, 32]) and
B = onehot(perm&63) moving ([64, 64]); partition p = k + 64*h holds row
half h so DVE builds use all 128 lanes while each matmul contracts 64
partitions at tile_position (64h, 32g).  Single bf16 pass (no hi/lo):
alpha rounding ~2e-3 rel, gate is 2e-2.

Key layout trick: one-hots are stored [p, i2_hi, digit, i2_lo] with the
LOW two i2 bits innermost (IL=4).  The is_equal in0 is then just ph/pl
viewed [p, i2h, 1->digit, i2l] — real data with stride-1 inner, so the
DVE runs in 2x mode with no host-side expansion — and the matmul reads
digit columns at stride 8B (2 per 16B SBUF line).  (Fully-strided PE
reads touch a new SBUF cacheline every element and throttle concurrent
DVE/ACT ops 2-3x; fully-contiguous reads need a 2MB host-expanded
input.  IL=4 gets both cheaply.)  Choosing the col-group g = i2_lo makes
psum partition (32g+jq) merge with the DRAM (row, jq) dims so each bank
drains in ONE 3-dim dma_start.  Matmuls stay in a single serialized PE
stream (h outer): concurrent col-group streams measure slower end-to-end
due to SBUF read pressure.  PSUM holds the whole 2MB per-core output;
ACT evacuates with the fused 1/S softmax normalization; all drain DMAs
issue from the sync HWDGE ring while evacs run on ACT.
"""

import os
import sys

sys.path.insert(0, "/opt/trn_rl_repo")

import numpy as np

N = 2048
K = 64
NCORES = 8
ROWS = N // NCORES          # 256 rows per core
Q = 32                      # stationary digit width (jq)
F = 64                      # moving digit width (jf)
CW = 32                     # i2 chunk width (4 chunks of 32)
IL = 4                      # low i2 bits kept innermost in one-hot layout

LAST_EXEC_NS = None
LAST_RESULTS = None

_cached = {}


def _build_bass():
    import concourse.tile as tile
    from concourse import bacc, mybir

    fp32 = mybir.dt.float32
    bf16 = mybir.dt.bfloat16
    i16 = mybir.dt.int16
    Copy = mybir.ActivationFunctionType.Copy
    Exp = mybir.ActivationFunctionType.Exp
    IsEq = mybir.AluOpType.is_equal

    nc = bacc.Bacc()

    ph_ext = nc.declare_dram_parameter("ph", [128, 128], i16, isOutput=False)
    bh_ext = nc.declare_dram_parameter("bhot", [128, 8192], bf16, isOutput=False)
    at_ext = nc.declare_dram_parameter("altp", [128, 2], fp32, isOutput=False)
    out_ext = nc.declare_dram_parameter("out", [ROWS, N], fp32, isOutput=True)

    with tile.TileContext(nc) as tc:
        with (
            tc.tile_pool(name="sbuf", bufs=1) as sb,
            tc.tile_pool(name="stage", bufs=10) as stp,
            tc.tile_pool(name="smax_psum", bufs=1, space="PSUM") as psmax,
            tc.tile_pool(name="psum", bufs=7, space="PSUM") as pp,
        ):
            # ---- input loads: 2 DMAs; iotas generated on-chip ---------------
            # One-hot tensors are laid out [p, i2h, digit, i2l] with the LOW
            # i2 bits innermost (IL=4): the is_equal in0 is just ph/pl viewed
            # [p, i2h, 1->digit, i2l] — real data, stride-1 inner, 2x DVE
            # mode with NO host expansion.  The matmul then reads digit cols
            # at stride IL*2 = 8B (2 per 16B SBUF line), cutting the line
            # thrash that throttles concurrent DVE/ACT ops.
            ph_t = sb.tile([128, 128], i16)
            at_t = sb.tile([128, 2], fp32)
            b_t = sb.tile([128, 32, F, IL], bf16)  # host one-hot [p,i2h,f,il]
            bh_v = bh_ext[:].rearrange(
                "p (ih f il) -> p ih f il", f=F, il=IL
            )
            # The B one-hot streams from DRAM in 4 chunks (the SDMA engines
            # are otherwise idle until the first bank drains), freeing the
            # DVE of its 4.5us is_equal build.  Chunks split across both
            # HWDGE rings — each ring runs DMAs ~serially end-to-end.
            nc.scalar.dma_start(out=ph_t[:], in_=ph_ext[:])
            nc.sync.dma_start(out=b_t[:, 0:8], in_=bh_v[:, 0:8])
            nc.scalar.dma_start(out=at_t[:], in_=at_ext[:])
            nc.sync.dma_start(out=b_t[:, 8:16], in_=bh_v[:, 8:16])
            nc.scalar.dma_start(out=b_t[:, 16:24], in_=bh_v[:, 16:24])
            nc.sync.dma_start(out=b_t[:, 24:32], in_=bh_v[:, 24:32])
            ph_v = ph_t[:].rearrange("p (ih il) -> p ih il", il=IL)
            iq_t = sb.tile([128, Q, IL], i16)   # [p, q, il] = q
            nc.gpsimd.iota(iq_t[:], pattern=[[1, Q], [0, IL]], channel_multiplier=0)
            al_t = at_t[:, 0:1]
            tp_t = at_t[:, 1:2]

            # ---- softmax head ----------------------------------------------
            # e = exp(alpha/T) unnormalized; S recovered via matmul with 0.5
            # (partitions hold k twice), 1/S applied at evacuation.
            rt_t = sb.tile([128, 1], fp32)
            e_t = sb.tile([128, 1], fp32)
            ln2_t = sb.tile([128, 1], fp32)
            prime_t = sb.tile([128, 1], fp32)
            half_col = sb.tile([128, 1], fp32)
            ones_row = sb.tile([1, 128], fp32)
            r_t = sb.tile([1, 1], fp32)
            rs_t = sb.tile([128, 1], fp32)
            scr_t = sb.tile([128, 512], bf16)   # warmup scratch (uninit junk)
            nc.vector.memset(scr_t[:, 0:2], 1.0)
            nc.vector.memset(ln2_t[:], float(np.log(2.0)))
            nc.vector.memset(half_col[:], 0.5)
            nc.vector.memset(ones_row[:], 1.0)
            # dep-free ACT op: hoists the one-time activation-table load off
            # the exp critical path.
            nc.scalar.activation(out=prime_t[:], in_=ln2_t[:], func=Exp)
            warm_ps = psmax.tile([1, 512], fp32, tag="smax")
            sum_ps = psmax.tile([1, 1], fp32, tag="smax")
            # HAM pre-warm: a few WIDE dep-free matmuls (~3us of PE busy in 7
            # instructions) so the clock gate reaches 8/8 before the real
            # stream without clogging the PE queue ahead of the softmax sum.
            for _ in range(7):
                nc.tensor.matmul(
                    warm_ps[:], lhsT=scr_t[:, 0:1], rhs=scr_t[:],
                    start=True, stop=True,
                )

            IH = 128 // IL                       # i2h extent (32)
            CH = IH // 4                         # i2h per chunk (8)
            a_t = sb.tile([128, IH, Q, IL], bf16)  # [p, i2h, jq, i2l]
            a_s = sb.tile([128, IH, Q, IL], bf16)  # alpha-scaled

            # DRAM view: row i = 32b + 4s + g, col j = q*64 + f;
            # psum partition = 32g + q, psum free = 64s + f.  With g the LOW
            # row bits, the DRAM dims (g, q) merge to one stride-64 dim, so
            # the whole bank drains in ONE 3-dim dma_start.
            oview = out_ext[:].rearrange(
                "(b s g) (q f) -> b g q s f", b=8, s=8, g=4, q=Q, f=F
            )

            banks = [None] * 8
            rb_ps = []

            def emit_builds(c, lo, ln):
                ih = slice(CH * c + lo, CH * c + lo + ln)
                nc.vector.tensor_tensor(
                    out=a_t[:, ih],
                    in0=ph_v[:, ih].unsqueeze(2).to_broadcast([128, ln, Q, IL]),
                    in1=iq_t[:].unsqueeze(1).to_broadcast([128, ln, Q, IL]),
                    op=IsEq,
                )
                return ih

            def emit_chunk(c):
                # (splitting the last chunk's builds to overlap banks 3/7's
                # first matmuls measured ~1.8us SLOWER — scheduler/sem churn)
                split3 = False
                ih = emit_builds(c, 0, 4 if split3 else CH)
                if c == 0:
                    # alpha chain after the builds (altp can land after ph;
                    # this keeps the DVE bubble-free) — exp runs on ACT in
                    # parallel so e_t is ready before a_s0 issues.
                    nc.vector.reciprocal(out=rt_t[:], in_=tp_t[:])
                    nc.scalar.activation(
                        out=e_t[:], in_=al_t[:], func=Exp, scale=rt_t[:]
                    )
                    nc.tensor.matmul(
                        sum_ps[:], lhsT=e_t[:], rhs=half_col[:],
                        start=True, stop=True,
                    )
                nc.vector.tensor_scalar(
                    out=a_s[:, ih], in0=a_t[:, ih], scalar1=e_t[:],
                    scalar2=None, op0=mybir.AluOpType.mult,
                )
                if c == 0:
                    nc.vector.reciprocal(out=r_t[:], in_=sum_ps[:])
                # Per h-half: 32 matmuls (h outer keeps the PE a single
                # serialized stream; g inner overlaps LDW), then that bank's
                # drain IMMEDIATELY — the evac's wait-sem then covers only
                # this bank's matmuls, so every bank drains ~1.2us earlier
                # than with a chunk-trailing drain.  Drain DMAs issue from
                # the sync ring (idle), evacs run on ACT.
                def emit_mms(srange):
                    for h in range(2):
                        kp = slice(64 * h, 64 * h + 64)
                        for s in srange:
                            for g in range(4):
                                # row r = 128h + i2, i2 = 4*(CH*c+s) + g
                                nc.tensor.matmul(
                                    banks[c + 4 * h][32 * g : 32 * g + 32, s],
                                    lhsT=a_s[kp, CH * c + s, :, g],
                                    rhs=b_t[kp, CH * c + s, :, g],
                                    start=True,
                                    stop=True,
                                    tile_position=(64 * h, 32 * g),
                                )

                for h in range(2):
                    banks[c + 4 * h] = pp.tile(
                        [128, 8, F], fp32, tag="bank", name=f"bank{c}_{h}"
                    )
                if split3:
                    emit_mms(range(4))
                    emit_builds(c, 4, 4)
                    nc.vector.tensor_scalar(
                        out=a_s[:, CH * c + 4 : CH * c + 8],
                        in0=a_t[:, CH * c + 4 : CH * c + 8], scalar1=e_t[:],
                        scalar2=None, op0=mybir.AluOpType.mult,
                    )
                    emit_mms(range(4, 8))
                    for h in range(2):
                        bi = c + 4 * h
                        stage = stp.tile(
                            [128, 8, F], fp32, tag="stage", name="stage"
                        )
                        nc.scalar.activation(
                            out=stage[:], in_=banks[bi][:], func=Copy,
                            scale=rs_t[:],
                        )
                        nc.sync.dma_start(out=oview[bi], in_=stage[:])
                    return
                for h in range(2):
                    bi = c + 4 * h
                    kp = slice(64 * h, 64 * h + 64)
                    for s in range(8):
                        for g in range(4):
                            # row r = 128h + i2, i2 = 4*(CH*c+s) + g
                            nc.tensor.matmul(
                                banks[bi][32 * g : 32 * g + 32, s],
                                lhsT=a_s[kp, CH * c + s, :, g],
                                rhs=b_t[kp, CH * c + s, :, g],
                                start=True,
                                stop=True,
                                tile_position=(64 * h, 32 * g),
                            )
                    if c == 0 and h == 0:
                        # softmax-tail broadcast BETWEEN bank 0's matmuls
                        # and its evac: placing it before the matmuls made
                        # them wait ~0.55us on r_t for nothing.
                        rb = psmax.tile(
                            [128, 1], fp32, tag="smax", name="rb_ps"
                        )
                        rb_ps.append(rb)
                        nc.tensor.matmul(
                            rb[:], lhsT=ones_row[:], rhs=r_t[:],
                            start=True, stop=True,
                        )
                        nc.vector.tensor_copy(out=rs_t[:], in_=rb[:])
                    stage = stp.tile(
                        [128, 8, F], fp32, tag="stage", name="stage"
                    )
                    nc.scalar.activation(
                        out=stage[:], in_=banks[bi][:], func=Copy,
                        scale=rs_t[:],
                    )
                    nc.sync.dma_start(out=oview[bi], in_=stage[:])

            for c in range(4):
                emit_chunk(c)

    if not nc.is_finalized():
        nc.finalize()
    return nc


def _prep_inputs(alpha_weights, perm_vectors, temperature):
    a = np.asarray(alpha_weights, dtype=np.float32).reshape(K)
    T = np.asarray(temperature, dtype=np.float32).reshape(())
    perm = np.asarray(perm_vectors).astype(np.int64).reshape(K, N)
    ph = (perm >> 6).astype(np.int16)
    pl = (perm & 63).astype(np.int16)
    al_t = np.concatenate([a, a])[:, None].copy()          # [128, 1]
    tp_t = np.full((128, 1), T, dtype=np.float32)
    import ml_dtypes

    fvals = np.arange(F, dtype=np.int16)
    in_maps = []
    for c in range(NCORES):
        # partition p = k + 64*h, column i2: row r = 128*h + i2 of this core
        phc = ph[:, c * ROWS : (c + 1) * ROWS].reshape(K, 2, 128)
        plc = pl[:, c * ROWS : (c + 1) * ROWS].reshape(K, 2, 128)
        plg = plc.transpose(1, 0, 2).reshape(128, 32, IL)    # [p, i2h, il]
        bhot = (plg[:, :, None, :] == fvals[None, None, :, None]).astype(
            ml_dtypes.bfloat16
        )                                                    # [p, i2h, f, il]
        in_maps.append(
            {
                "ph": phc.transpose(1, 0, 2).reshape(128, 128).copy(),
                "bhot": bhot.reshape(128, 8192).copy(),
                "altp": np.concatenate([al_t, tp_t], axis=1).copy(),
            }
        )
    return in_maps


def _install_ntff_hook():
    """Provide antenv.axon_hooks (missing in this image) so that
    run_bass_kernel_spmd(trace=True) can capture NTFF profiles via the
    axon PJRT .so (same mechanism as trn_agent_boot.trn_boot)."""
    import contextlib
    import ctypes
    import types

    try:
        from antenv.axon_hooks import get_axon_ntff_profile_hook  # noqa: F401

        return True
    except ImportError:
        pass
    so_path = "/opt/axon/libaxon_pjrt.so"
    if not os.path.exists(so_path):
        return False
    lib = ctypes.CDLL(so_path)
    if not hasattr(lib, "axon_start_nrt_profile"):
        return False
    lib.axon_start_nrt_profile.argtypes = [
        ctypes.POINTER(ctypes.c_int64),
        ctypes.c_size_t,
    ]
    lib.axon_start_nrt_profile.restype = ctypes.c_int64
    lib.axon_stop_nrt_profile.argtypes = [ctypes.c_char_p]
    lib.axon_stop_nrt_profile.restype = ctypes.c_int64

    @contextlib.contextmanager
    def _hook(output_dir, device_ids):
        import jax

        jax.devices()
        if device_ids:
            ids = (ctypes.c_int64 * len(device_ids))(*device_ids)
            rc = lib.axon_start_nrt_profile(ids, len(device_ids))
        else:
            rc = lib.axon_start_nrt_profile(None, 0)
        if rc != 0:
            raise RuntimeError(f"axon_start_nrt_profile rc={rc}")
        try:
            yield
        finally:
            n = lib.axon_stop_nrt_profile(str(output_dir).encode())
            print(f"ntff profile: {n} file(s) written to {output_dir}")

    import antenv

    mod = types.ModuleType("antenv.axon_hooks")
    mod.get_axon_ntff_profile_hook = lambda: _hook
    mod.set_axon_ntff_profile_hook = lambda h: None
    sys.modules["antenv.axon_hooks"] = mod
    antenv.axon_hooks = mod
    return True


def kernel(alpha_weights, perm_vectors, temperature):
    global LAST_EXEC_NS, LAST_RESULTS
    from concourse.bass_utils import run_bass_kernel_spmd

    if "nc" not in _cached:
        _cached["nc"] = _build_bass()
    nc = _cached["nc"]
    in_maps = _prep_inputs(alpha_weights, perm_vectors, temperature)
    core_ids = list(range(NCORES))
    trace = os.environ.get("KERNEL_TRACE", "0") == "1"
    if trace:
        trace = _install_ntff_hook()
    try:
        res = run_bass_kernel_spmd(nc, in_maps, core_ids, trace=trace)
    except Exception:
        if not trace:
            raise
        res = run_bass_kernel_spmd(nc, in_maps, core_ids, trace=False)
    LAST_EXEC_NS = res.exec_time_ns
    LAST_RESULTS = res
    out = np.concatenate([res.results[c]["out"] for c in range(NCORES)], axis=0)
    return out.astype(np.float32)


if __name__ == "__main__":
    rng = np.random.default_rng(0)
    a = rng.standard_normal(K).astype(np.float32)
    perm = np.stack([rng.permutation(N) for _ in range(K)]).astype(np.int64)
    T = np.ones((), np.float32)
    out = kernel(a, perm, T)
    # numpy reference
    al = np.exp(a / T - (a / T).max())
    al /= al.sum()
    exp = np.zeros((N, N), np.float32)
    np.add.at(exp, (np.broadcast_to(np.arange(N), (K, N)), perm), al[:, None])
    print("max abs err:", np.abs(out - exp).max(), "max ref:", np.abs(exp).max())
    print("exec ns:", LAST_EXEC_NS)


# revision 101
# speedup vs baseline: 1.2546x; 1.2546x over previous
"""AlphaPermutationLayer Trainium2 kernel.

out[i, j] = sum_k softmax(alpha/T)[k] * (perm[k, i] == j),  N=2048, K=64.

Sharding: output ROWS across the 8 cores (row i depends only on perm[:, i]
and alpha — no collective).  Per core (256 rows), digit-split
j = jq*64 + jf (jq in [0,32), jf in [0,64)); one matmul per row i:
    out_i[jq, jf] = sum_k A_i[k, jq] * B_i[k, jf]
with A = alpha-scaled onehot(perm>>6) stationary ([64, 32]) and
B = onehot(perm&63) moving ([64, 64]); partition p = k + 64*h holds row
half h so DVE builds use all 128 lanes while each matmul contracts 64
partitions at tile_position (64h, 32g).  Single bf16 pass (no hi/lo):
alpha rounding ~2e-3 rel, gate is 2e-2.

Key layout trick: one-hots are stored [p, i2_hi, digit, i2_lo] with the
LOW two i2 bits innermost (IL=4).  The is_equal in0 is then just ph/pl
viewed [p, i2h, 1->digit, i2l] — real data with stride-1 inner, so the
DVE runs in 2x mode with no host-side expansion — and the matmul reads
digit columns at stride 8B (2 per 16B SBUF line).  (Fully-strided PE
reads touch a new SBUF cacheline every element and throttle concurrent
DVE/ACT ops 2-3x; fully-contiguous reads need a 2MB host-expanded
input.  IL=4 gets both cheaply.)  Choosing the col-group g = i2_lo makes
psum partition (32g+jq) merge with the DRAM (row, jq) dims so each bank
drains in ONE 3-dim dma_start.  Matmuls stay in a single serialized PE
stream (h outer): concurrent col-group streams measure slower end-to-end
due to SBUF read pressure.  PSUM holds the whole 2MB per-core output;
ACT evacuates with the fused 1/S softmax normalization; all drain DMAs
issue from the sync HWDGE ring while evacs run on ACT.
"""

import os
import sys

sys.path.insert(0, "/opt/trn_rl_repo")

import numpy as np

N = 2048
K = 64
NCORES = 8
ROWS = N // NCORES          # 256 rows per core
Q = 32                      # stationary digit width (jq)
F = 64                      # moving digit width (jf)
CW = 32                     # i2 chunk width (4 chunks of 32)
IL = 4                      # low i2 bits kept innermost in one-hot layout

LAST_EXEC_NS = None
LAST_RESULTS = None

_cached = {}


def _build_bass():
    import concourse.tile as tile
    from concourse import bacc, mybir

    fp32 = mybir.dt.float32
    bf16 = mybir.dt.bfloat16
    i16 = mybir.dt.int16
    Copy = mybir.ActivationFunctionType.Copy
    Exp = mybir.ActivationFunctionType.Exp
    IsEq = mybir.AluOpType.is_equal

    nc = bacc.Bacc()

    ph_ext = nc.declare_dram_parameter("ph", [128, 128], i16, isOutput=False)
    pl_ext = nc.declare_dram_parameter("pl", [128, 128], i16, isOutput=False)
    at_ext = nc.declare_dram_parameter("altp", [128, 2], fp32, isOutput=False)
    out_ext = nc.declare_dram_parameter("out", [ROWS, N], fp32, isOutput=True)

    with tile.TileContext(nc) as tc:
        with (
            tc.tile_pool(name="sbuf", bufs=1) as sb,
            tc.tile_pool(name="stage", bufs=10) as stp,
            tc.tile_pool(name="smax_psum", bufs=1, space="PSUM") as psmax,
            tc.tile_pool(name="psum", bufs=7, space="PSUM") as pp,
        ):
            # ---- input loads: 2 DMAs; iotas generated on-chip ---------------
            # One-hot tensors are laid out [p, i2h, digit, i2l] with the LOW
            # i2 bits innermost (IL=4): the is_equal in0 is just ph/pl viewed
            # [p, i2h, 1->digit, i2l] — real data, stride-1 inner, 2x DVE
            # mode with NO host expansion.  The matmul then reads digit cols
            # at stride IL*2 = 8B (2 per 16B SBUF line), cutting the line
            # thrash that throttles concurrent DVE/ACT ops.
            ph_t = sb.tile([128, 128], i16)
            pl_t = sb.tile([128, 128], i16)
            at_t = sb.tile([128, 2], fp32)
            nc.sync.dma_start(out=ph_t[:, 0:64], in_=ph_ext[:, 0:64])
            nc.scalar.dma_start(out=ph_t[:, 64:128], in_=ph_ext[:, 64:128])
            nc.sync.dma_start(out=pl_t[:], in_=pl_ext[:])
            nc.scalar.dma_start(out=at_t[:], in_=at_ext[:])
            ph_v = ph_t[:].rearrange("p (ih il) -> p ih il", il=IL)
            pl_v = pl_t[:].rearrange("p (ih il) -> p ih il", il=IL)
            iq_t = sb.tile([128, Q, IL], i16)   # [p, q, il] = q
            if_t = sb.tile([128, F, IL], i16)   # [p, f, il] = f
            nc.gpsimd.iota(iq_t[:], pattern=[[1, Q], [0, IL]], channel_multiplier=0)
            nc.gpsimd.iota(if_t[:], pattern=[[1, F], [0, IL]], channel_multiplier=0)
            al_t = at_t[:, 0:1]
            tp_t = at_t[:, 1:2]

            # ---- softmax head ----------------------------------------------
            # e = exp(alpha/T) unnormalized; S recovered via matmul with 0.5
            # (partitions hold k twice), 1/S applied at evacuation.
            rt_t = sb.tile([128, 1], fp32)
            e_t = sb.tile([128, 1], fp32)
            ln2_t = sb.tile([128, 1], fp32)
            prime_t = sb.tile([128, 1], fp32)
            half_col = sb.tile([128, 1], fp32)
            ones_row = sb.tile([1, 128], fp32)
            r_t = sb.tile([1, 1], fp32)
            rs_t = sb.tile([128, 1], fp32)
            scr_t = sb.tile([128, 512], bf16)   # warmup scratch (uninit junk)
            nc.vector.memset(scr_t[:, 0:2], 1.0)
            nc.vector.memset(ln2_t[:], float(np.log(2.0)))
            nc.vector.memset(half_col[:], 0.5)
            nc.vector.memset(ones_row[:], 1.0)
            # dep-free ACT op: hoists the one-time activation-table load off
            # the exp critical path.
            nc.scalar.activation(out=prime_t[:], in_=ln2_t[:], func=Exp)
            warm_ps = psmax.tile([1, 512], fp32, tag="smax")
            sum_ps = psmax.tile([1, 1], fp32, tag="smax")
            # HAM pre-warm: a few WIDE dep-free matmuls (~3us of PE busy in 7
            # instructions) so the clock gate reaches 8/8 before the real
            # stream without clogging the PE queue ahead of the softmax sum.
            for _ in range(7):
                nc.tensor.matmul(
                    warm_ps[:], lhsT=scr_t[:, 0:1], rhs=scr_t[:],
                    start=True, stop=True,
                )

            IH = 128 // IL                       # i2h extent (32)
            CH = IH // 4                         # i2h per chunk (8)
            a_t = sb.tile([128, IH, Q, IL], bf16)  # [p, i2h, jq, i2l]
            a_s = sb.tile([128, IH, Q, IL], bf16)  # alpha-scaled
            b_t = sb.tile([128, IH, F, IL], bf16)  # [p, i2h, jf, i2l]

            # DRAM view: row i = 32b + 4s + g, col j = q*64 + f;
            # psum partition = 32g + q, psum free = 64s + f.  With g the LOW
            # row bits, the DRAM dims (g, q) merge to one stride-64 dim, so
            # the whole bank drains in ONE 3-dim dma_start.
            oview = out_ext[:].rearrange(
                "(b s g) (q f) -> b g q s f", b=8, s=8, g=4, q=Q, f=F
            )

            banks = [None] * 8
            rb_ps = []

            def emit_builds(c, lo, ln):
                ih = slice(CH * c + lo, CH * c + lo + ln)
                nc.vector.tensor_tensor(
                    out=a_t[:, ih],
                    in0=ph_v[:, ih].unsqueeze(2).to_broadcast([128, ln, Q, IL]),
                    in1=iq_t[:].unsqueeze(1).to_broadcast([128, ln, Q, IL]),
                    op=IsEq,
                )
                nc.vector.tensor_tensor(
                    out=b_t[:, ih],
                    in0=pl_v[:, ih].unsqueeze(2).to_broadcast([128, ln, F, IL]),
                    in1=if_t[:].unsqueeze(1).to_broadcast([128, ln, F, IL]),
                    op=IsEq,
                )
                return ih

            def emit_chunk(c):
                # (splitting the last chunk's builds to overlap banks 3/7's
                # first matmuls measured ~1.8us SLOWER — scheduler/sem churn)
                split3 = False
                ih = emit_builds(c, 0, 4 if split3 else CH)
                if c == 0:
                    # alpha chain after the builds (altp can land after ph;
                    # this keeps the DVE bubble-free) — exp runs on ACT in
                    # parallel so e_t is ready before a_s0 issues.
                    nc.vector.reciprocal(out=rt_t[:], in_=tp_t[:])
                    nc.scalar.activation(
                        out=e_t[:], in_=al_t[:], func=Exp, scale=rt_t[:]
                    )
                    nc.tensor.matmul(
                        sum_ps[:], lhsT=e_t[:], rhs=half_col[:],
                        start=True, stop=True,
                    )
                nc.vector.tensor_scalar(
                    out=a_s[:, ih], in0=a_t[:, ih], scalar1=e_t[:],
                    scalar2=None, op0=mybir.AluOpType.mult,
                )
                if c == 0:
                    nc.vector.reciprocal(out=r_t[:], in_=sum_ps[:])
                # Per h-half: 32 matmuls (h outer keeps the PE a single
                # serialized stream; g inner overlaps LDW), then that bank's
                # drain IMMEDIATELY — the evac's wait-sem then covers only
                # this bank's matmuls, so every bank drains ~1.2us earlier
                # than with a chunk-trailing drain.  Drain DMAs issue from
                # the sync ring (idle), evacs run on ACT.
                def emit_mms(srange):
                    for h in range(2):
                        kp = slice(64 * h, 64 * h + 64)
                        for s in srange:
                            for g in range(4):
                                # row r = 128h + i2, i2 = 4*(CH*c+s) + g
                                nc.tensor.matmul(
                                    banks[c + 4 * h][32 * g : 32 * g + 32, s],
                                    lhsT=a_s[kp, CH * c + s, :, g],
                                    rhs=b_t[kp, CH * c + s, :, g],
                                    start=True,
                                    stop=True,
                                    tile_position=(64 * h, 32 * g),
                                )

                for h in range(2):
                    banks[c + 4 * h] = pp.tile(
                        [128, 8, F], fp32, tag="bank", name=f"bank{c}_{h}"
                    )
                if split3:
                    emit_mms(range(4))
                    emit_builds(c, 4, 4)
                    nc.vector.tensor_scalar(
                        out=a_s[:, CH * c + 4 : CH * c + 8],
                        in0=a_t[:, CH * c + 4 : CH * c + 8], scalar1=e_t[:],
                        scalar2=None, op0=mybir.AluOpType.mult,
                    )
                    emit_mms(range(4, 8))
                    for h in range(2):
                        bi = c + 4 * h
                        stage = stp.tile(
                            [128, 8, F], fp32, tag="stage", name="stage"
                        )
                        nc.scalar.activation(
                            out=stage[:], in_=banks[bi][:], func=Copy,
                            scale=rs_t[:],
                        )
                        nc.sync.dma_start(out=oview[bi], in_=stage[:])
                    return
                for h in range(2):
                    bi = c + 4 * h
                    kp = slice(64 * h, 64 * h + 64)
                    for s in range(8):
                        for g in range(4):
                            # row r = 128h + i2, i2 = 4*(CH*c+s) + g
                            nc.tensor.matmul(
                                banks[bi][32 * g : 32 * g + 32, s],
                                lhsT=a_s[kp, CH * c + s, :, g],
                                rhs=b_t[kp, CH * c + s, :, g],
                                start=True,
                                stop=True,
                                tile_position=(64 * h, 32 * g),
                            )
                    if c == 0 and h == 0:
                        # softmax-tail broadcast BETWEEN bank 0's matmuls
                        # and its evac: placing it before the matmuls made
                        # them wait ~0.55us on r_t for nothing.
                        rb = psmax.tile(
                            [128, 1], fp32, tag="smax", name="rb_ps"
                        )
                        rb_ps.append(rb)
                        nc.tensor.matmul(
                            rb[:], lhsT=ones_row[:], rhs=r_t[:],
                            start=True, stop=True,
                        )
                        nc.vector.tensor_copy(out=rs_t[:], in_=rb[:])
                    stage = stp.tile(
                        [128, 8, F], fp32, tag="stage", name="stage"
                    )
                    nc.scalar.activation(
                        out=stage[:], in_=banks[bi][:], func=Copy,
                        scale=rs_t[:],
                    )
                    nc.sync.dma_start(out=oview[bi], in_=stage[:])

            for c in range(4):
                emit_chunk(c)

    if not nc.is_finalized():
        nc.finalize()
    return nc


def _prep_inputs(alpha_weights, perm_vectors, temperature):
    a = np.asarray(alpha_weights, dtype=np.float32).reshape(K)
    T = np.asarray(temperature, dtype=np.float32).reshape(())
    perm = np.asarray(perm_vectors).astype(np.int64).reshape(K, N)
    ph = (perm >> 6).astype(np.int16)
    pl = (perm & 63).astype(np.int16)
    al_t = np.concatenate([a, a])[:, None].copy()          # [128, 1]
    tp_t = np.full((128, 1), T, dtype=np.float32)
    in_maps = []
    for c in range(NCORES):
        # partition p = k + 64*h, column i2: row r = 128*h + i2 of this core
        phc = ph[:, c * ROWS : (c + 1) * ROWS].reshape(K, 2, 128)
        plc = pl[:, c * ROWS : (c + 1) * ROWS].reshape(K, 2, 128)
        in_maps.append(
            {
                "ph": phc.transpose(1, 0, 2).reshape(128, 128).copy(),
                "pl": plc.transpose(1, 0, 2).reshape(128, 128).copy(),
                "altp": np.concatenate([al_t, tp_t], axis=1).copy(),
            }
        )
    return in_maps


def _install_ntff_hook():
    """Provide antenv.axon_hooks (missing in this image) so that
    run_bass_kernel_spmd(trace=True) can capture NTFF profiles via the
    axon PJRT .so (same mechanism as trn_agent_boot.trn_boot)."""
    import contextlib
    import ctypes
    import types

    try:
        from antenv.axon_hooks import get_axon_ntff_profile_hook  # noqa: F401

        return True
    except ImportError:
        pass
    so_path = "/opt/axon/libaxon_pjrt.so"
    if not os.path.exists(so_path):
        return False
    lib = ctypes.CDLL(so_path)
    if not hasattr(lib, "axon_start_nrt_profile"):
        return False
    lib.axon_start_nrt_profile.argtypes = [
        ctypes.POINTER(ctypes.c_int64),
        ctypes.c_size_t,
    ]
    lib.axon_start_nrt_profile.restype = ctypes.c_int64
    lib.axon_stop_nrt_profile.argtypes = [ctypes.c_char_p]
    lib.axon_stop_nrt_profile.restype = ctypes.c_int64

    @contextlib.contextmanager
    def _hook(output_dir, device_ids):
        import jax

        jax.devices()
        if device_ids:
            ids = (ctypes.c_int64 * len(device_ids))(*device_ids)
            rc = lib.axon_start_nrt_profile(ids, len(device_ids))
        else:
            rc = lib.axon_start_nrt_profile(None, 0)
        if rc != 0:
            raise RuntimeError(f"axon_start_nrt_profile rc={rc}")
        try:
            yield
        finally:
            n = lib.axon_stop_nrt_profile(str(output_dir).encode())
            print(f"ntff profile: {n} file(s) written to {output_dir}")

    import antenv

    mod = types.ModuleType("antenv.axon_hooks")
    mod.get_axon_ntff_profile_hook = lambda: _hook
    mod.set_axon_ntff_profile_hook = lambda h: None
    sys.modules["antenv.axon_hooks"] = mod
    antenv.axon_hooks = mod
    return True


def kernel(alpha_weights, perm_vectors, temperature):
    global LAST_EXEC_NS, LAST_RESULTS
    from concourse.bass_utils import run_bass_kernel_spmd

    if "nc" not in _cached:
        _cached["nc"] = _build_bass()
    nc = _cached["nc"]
    in_maps = _prep_inputs(alpha_weights, perm_vectors, temperature)
    core_ids = list(range(NCORES))
    trace = os.environ.get("KERNEL_TRACE", "0") == "1"
    if trace:
        trace = _install_ntff_hook()
    try:
        res = run_bass_kernel_spmd(nc, in_maps, core_ids, trace=trace)
    except Exception:
        if not trace:
            raise
        res = run_bass_kernel_spmd(nc, in_maps, core_ids, trace=False)
    LAST_EXEC_NS = res.exec_time_ns
    LAST_RESULTS = res
    out = np.concatenate([res.results[c]["out"] for c in range(NCORES)], axis=0)
    return out.astype(np.float32)


if __name__ == "__main__":
    rng = np.random.default_rng(0)
    a = rng.standard_normal(K).astype(np.float32)
    perm = np.stack([rng.permutation(N) for _ in range(K)]).astype(np.int64)
    T = np.ones((), np.float32)
    out = kernel(a, perm, T)
    # numpy reference
    al = np.exp(a / T - (a / T).max())
    al /= al.sum()
    exp = np.zeros((N, N), np.float32)
    np.add.at(exp, (np.broadcast_to(np.arange(N), (K, N)), perm), al[:, None])
    print("max abs err:", np.abs(out - exp).max(), "max ref:", np.abs(exp).max())
    print("exec ns:", LAST_EXEC_NS)


# revision 104
# speedup vs baseline: 1.2587x; 1.0033x over previous
"""AlphaPermutationLayer Trainium2 kernel.

out[i, j] = sum_k softmax(alpha/T)[k] * (perm[k, i] == j),  N=2048, K=64.

Sharding: output ROWS across the 8 cores (row i depends only on perm[:, i]
and alpha — no collective).  Per core (256 rows), digit-split
j = jq*64 + jf (jq in [0,32), jf in [0,64)); one matmul per row i:
    out_i[jq, jf] = sum_k A_i[k, jq] * B_i[k, jf]
with A = alpha-scaled onehot(perm>>6) stationary ([64, 32]) and
B = onehot(perm&63) moving ([64, 64]); partition p = k + 64*h holds row
half h so DVE builds use all 128 lanes while each matmul contracts 64
partitions at tile_position (64h, 32g).  Single bf16 pass (no hi/lo):
alpha rounding ~2e-3 rel, gate is 2e-2.

Key layout trick: one-hots are stored [p, i2_hi, digit, i2_lo] with the
LOW two i2 bits innermost (IL=4).  The is_equal in0 is then just ph/pl
viewed [p, i2h, 1->digit, i2l] — real data with stride-1 inner, so the
DVE runs in 2x mode with no host-side expansion — and the matmul reads
digit columns at stride 8B (2 per 16B SBUF line).  (Fully-strided PE
reads touch a new SBUF cacheline every element and throttle concurrent
DVE/ACT ops 2-3x; fully-contiguous reads need a 2MB host-expanded
input.  IL=4 gets both cheaply.)  Choosing the col-group g = i2_lo makes
psum partition (32g+jq) merge with the DRAM (row, jq) dims so each bank
drains in ONE 3-dim dma_start.  Matmuls stay in a single serialized PE
stream (h outer): concurrent col-group streams measure slower end-to-end
due to SBUF read pressure.  PSUM holds the whole 2MB per-core output;
ACT evacuates with the fused 1/S softmax normalization; all drain DMAs
issue from the sync HWDGE ring while evacs run on ACT.
"""

import os
import sys

sys.path.insert(0, "/opt/trn_rl_repo")

import numpy as np

N = 2048
K = 64
NCORES = 8
ROWS = N // NCORES          # 256 rows per core
Q = 32                      # stationary digit width (jq)
F = 64                      # moving digit width (jf)
CW = 32                     # i2 chunk width (4 chunks of 32)
IL = 4                      # low i2 bits kept innermost in one-hot layout

LAST_EXEC_NS = None
LAST_RESULTS = None

_cached = {}


def _build_bass():
    import concourse.tile as tile
    from concourse import bacc, mybir

    fp32 = mybir.dt.float32
    bf16 = mybir.dt.bfloat16
    i16 = mybir.dt.int16
    Copy = mybir.ActivationFunctionType.Copy
    Exp = mybir.ActivationFunctionType.Exp
    IsEq = mybir.AluOpType.is_equal

    nc = bacc.Bacc()

    ph_ext = nc.declare_dram_parameter("ph", [128, 128], i16, isOutput=False)
    pl_ext = nc.declare_dram_parameter("pl", [128, 128], i16, isOutput=False)
    at_ext = nc.declare_dram_parameter("altp", [128, 2], fp32, isOutput=False)
    out_ext = nc.declare_dram_parameter("out", [ROWS, N], fp32, isOutput=True)

    with tile.TileContext(nc) as tc:
        with (
            tc.tile_pool(name="sbuf", bufs=1) as sb,
            tc.tile_pool(name="stage", bufs=10) as stp,
            tc.tile_pool(name="smax_psum", bufs=1, space="PSUM") as psmax,
            tc.tile_pool(name="psum", bufs=7, space="PSUM") as pp,
        ):
            # ---- input loads: 2 DMAs; iotas generated on-chip ---------------
            # One-hot tensors are laid out [p, i2h, digit, i2l] with the LOW
            # i2 bits innermost (IL=4): the is_equal in0 is just ph/pl viewed
            # [p, i2h, 1->digit, i2l] — real data, stride-1 inner, 2x DVE
            # mode with NO host expansion.  The matmul then reads digit cols
            # at stride IL*2 = 8B (2 per 16B SBUF line), cutting the line
            # thrash that throttles concurrent DVE/ACT ops.
            ph_t = sb.tile([128, 128], i16)
            pl_t = sb.tile([128, 128], i16)
            at_t = sb.tile([128, 2], fp32)
            nc.sync.dma_start(out=ph_t[:, 0:64], in_=ph_ext[:, 0:64])
            nc.scalar.dma_start(out=ph_t[:, 64:128], in_=ph_ext[:, 64:128])
            nc.sync.dma_start(out=pl_t[:], in_=pl_ext[:])
            nc.scalar.dma_start(out=at_t[:], in_=at_ext[:])
            ph_v = ph_t[:].rearrange("p (ih il) -> p ih il", il=IL)
            pl_v = pl_t[:].rearrange("p (ih il) -> p ih il", il=IL)
            iq_t = sb.tile([128, Q, IL], i16)   # [p, q, il] = q
            if_t = sb.tile([128, F, IL], i16)   # [p, f, il] = f
            nc.gpsimd.iota(iq_t[:], pattern=[[1, Q], [0, IL]], channel_multiplier=0)
            nc.gpsimd.iota(if_t[:], pattern=[[1, F], [0, IL]], channel_multiplier=0)
            al_t = at_t[:, 0:1]
            tp_t = at_t[:, 1:2]

            # ---- softmax head ----------------------------------------------
            # e = exp(alpha/T) unnormalized; S recovered via matmul with 0.5
            # (partitions hold k twice), 1/S applied at evacuation.
            rt_t = sb.tile([128, 1], fp32)
            e_t = sb.tile([128, 1], fp32)
            ln2_t = sb.tile([128, 1], fp32)
            prime_t = sb.tile([128, 1], fp32)
            half_col = sb.tile([128, 1], fp32)
            ones_row = sb.tile([1, 128], fp32)
            r_t = sb.tile([1, 1], fp32)
            rs_t = sb.tile([128, 1], fp32)
            scr_t = sb.tile([128, 512], bf16)   # warmup scratch (uninit junk)
            nc.vector.memset(scr_t[:, 0:2], 1.0)
            nc.vector.memset(ln2_t[:], float(np.log(2.0)))
            nc.vector.memset(half_col[:], 0.5)
            nc.vector.memset(ones_row[:], 1.0)
            # dep-free ACT op: hoists the one-time activation-table load off
            # the exp critical path.
            nc.scalar.activation(out=prime_t[:], in_=ln2_t[:], func=Exp)
            warm_ps = psmax.tile([1, 512], fp32, tag="smax")
            sum_ps = psmax.tile([1, 1], fp32, tag="smax")
            # HAM pre-warm: a few WIDE dep-free matmuls (~3us of PE busy in 7
            # instructions) so the clock gate reaches 8/8 before the real
            # stream without clogging the PE queue ahead of the softmax sum.
            for _ in range(7):
                nc.tensor.matmul(
                    warm_ps[:], lhsT=scr_t[:, 0:1], rhs=scr_t[:],
                    start=True, stop=True,
                )

            IH = 128 // IL                       # i2h extent (32)
            CH = IH // 4                         # i2h per chunk (8)
            d_t = sb.tile([128, IH, Q, IL], i16)   # ph - jq  (0 at the hot q)
            a_s = sb.tile([128, IH, Q, IL], bf16)  # alpha-scaled one-hot
            b_t = sb.tile([128, IH, F, IL], bf16)  # [p, i2h, jf, i2l]

            # DRAM view: row i = 32b + 4s + g, col j = q*64 + f;
            # psum partition = 32g + q, psum free = 64s + f.  With g the LOW
            # row bits, the DRAM dims (g, q) merge to one stride-64 dim, so
            # the whole bank drains in ONE 3-dim dma_start.
            oview = out_ext[:].rearrange(
                "(b s g) (q f) -> b g q s f", b=8, s=8, g=4, q=Q, f=F
            )

            banks = [None] * 8
            rb_ps = []

            def emit_builds(c, lo, ln):
                ih = slice(CH * c + lo, CH * c + lo + ln)
                # subtract instead of is_equal (same 2x cost), so the scale
                # pass can FUSE compare+scale: a_s = (d==0)*e in one 4x
                # tensor_scalar — drops the separate one-hot intermediate.
                nc.vector.tensor_tensor(
                    out=d_t[:, ih],
                    in0=ph_v[:, ih].unsqueeze(2).to_broadcast([128, ln, Q, IL]),
                    in1=iq_t[:].unsqueeze(1).to_broadcast([128, ln, Q, IL]),
                    op=mybir.AluOpType.subtract,
                )
                nc.vector.tensor_tensor(
                    out=b_t[:, ih],
                    in0=pl_v[:, ih].unsqueeze(2).to_broadcast([128, ln, F, IL]),
                    in1=if_t[:].unsqueeze(1).to_broadcast([128, ln, F, IL]),
                    op=IsEq,
                )
                return ih

            def emit_chunk(c):
                # (splitting the last chunk's builds to overlap banks 3/7's
                # first matmuls measured ~1.8us SLOWER — scheduler/sem churn)
                split3 = False
                ih = emit_builds(c, 0, 4 if split3 else CH)
                if c == 0:
                    # alpha chain after the builds (altp can land after ph;
                    # this keeps the DVE bubble-free) — exp runs on ACT in
                    # parallel so e_t is ready before a_s0 issues.
                    nc.vector.reciprocal(out=rt_t[:], in_=tp_t[:])
                    nc.scalar.activation(
                        out=e_t[:], in_=al_t[:], func=Exp, scale=rt_t[:]
                    )
                    nc.tensor.matmul(
                        sum_ps[:], lhsT=e_t[:], rhs=half_col[:],
                        start=True, stop=True,
                    )
                nc.vector.tensor_scalar(
                    out=a_s[:, ih], in0=d_t[:, ih], scalar1=0.0,
                    scalar2=e_t[:], op0=IsEq, op1=mybir.AluOpType.mult,
                )
                if c == 0:
                    nc.vector.reciprocal(out=r_t[:], in_=sum_ps[:])
                # Per h-half: 32 matmuls (h outer keeps the PE a single
                # serialized stream; g inner overlaps LDW), then that bank's
                # drain IMMEDIATELY — the evac's wait-sem then covers only
                # this bank's matmuls, so every bank drains ~1.2us earlier
                # than with a chunk-trailing drain.  Drain DMAs issue from
                # the sync ring (idle), evacs run on ACT.
                def emit_mms(srange):
                    for h in range(2):
                        kp = slice(64 * h, 64 * h + 64)
                        for s in srange:
                            for g in range(4):
                                # row r = 128h + i2, i2 = 4*(CH*c+s) + g
                                nc.tensor.matmul(
                                    banks[c + 4 * h][32 * g : 32 * g + 32, s],
                                    lhsT=a_s[kp, CH * c + s, :, g],
                                    rhs=b_t[kp, CH * c + s, :, g],
                                    start=True,
                                    stop=True,
                                    tile_position=(64 * h, 32 * g),
                                )

                for h in range(2):
                    banks[c + 4 * h] = pp.tile(
                        [128, 8, F], fp32, tag="bank", name=f"bank{c}_{h}"
                    )
                if split3:
                    emit_mms(range(4))
                    emit_builds(c, 4, 4)
                    nc.vector.tensor_scalar(
                        out=a_s[:, CH * c + 4 : CH * c + 8],
                        in0=a_t[:, CH * c + 4 : CH * c + 8], scalar1=e_t[:],
                        scalar2=None, op0=mybir.AluOpType.mult,
                    )
                    emit_mms(range(4, 8))
                    for h in range(2):
                        bi = c + 4 * h
                        stage = stp.tile(
                            [128, 8, F], fp32, tag="stage", name="stage"
                        )
                        nc.scalar.activation(
                            out=stage[:], in_=banks[bi][:], func=Copy,
                            scale=rs_t[:],
                        )
                        nc.sync.dma_start(out=oview[bi], in_=stage[:])
                    return
                for h in range(2):
                    bi = c + 4 * h
                    kp = slice(64 * h, 64 * h + 64)
                    for s in range(8):
                        for g in range(4):
                            # row r = 128h + i2, i2 = 4*(CH*c+s) + g
                            nc.tensor.matmul(
                                banks[bi][32 * g : 32 * g + 32, s],
                                lhsT=a_s[kp, CH * c + s, :, g],
                                rhs=b_t[kp, CH * c + s, :, g],
                                start=True,
                                stop=True,
                                tile_position=(64 * h, 32 * g),
                            )
                    if c == 0 and h == 0:
                        # softmax-tail broadcast BETWEEN bank 0's matmuls
                        # and its evac: placing it before the matmuls made
                        # them wait ~0.55us on r_t for nothing.
                        rb = psmax.tile(
                            [128, 1], fp32, tag="smax", name="rb_ps"
                        )
                        rb_ps.append(rb)
                        nc.tensor.matmul(
                            rb[:], lhsT=ones_row[:], rhs=r_t[:],
                            start=True, stop=True,
                        )
                        nc.vector.tensor_copy(out=rs_t[:], in_=rb[:])
                    stage = stp.tile(
                        [128, 8, F], fp32, tag="stage", name="stage"
                    )
                    nc.scalar.activation(
                        out=stage[:], in_=banks[bi][:], func=Copy,
                        scale=rs_t[:],
                    )
                    nc.sync.dma_start(out=oview[bi], in_=stage[:])

            for c in range(4):
                emit_chunk(c)

    if not nc.is_finalized():
        nc.finalize()
    return nc


def _prep_inputs(alpha_weights, perm_vectors, temperature):
    a = np.asarray(alpha_weights, dtype=np.float32).reshape(K)
    T = np.asarray(temperature, dtype=np.float32).reshape(())
    perm = np.asarray(perm_vectors).astype(np.int64).reshape(K, N)
    ph = (perm >> 6).astype(np.int16)
    pl = (perm & 63).astype(np.int16)
    al_t = np.concatenate([a, a])[:, None].copy()          # [128, 1]
    tp_t = np.full((128, 1), T, dtype=np.float32)
    in_maps = []
    for c in range(NCORES):
        # partition p = k + 64*h, column i2: row r = 128*h + i2 of this core
        phc = ph[:, c * ROWS : (c + 1) * ROWS].reshape(K, 2, 128)
        plc = pl[:, c * ROWS : (c + 1) * ROWS].reshape(K, 2, 128)
        in_maps.append(
            {
                "ph": phc.transpose(1, 0, 2).reshape(128, 128).copy(),
                "pl": plc.transpose(1, 0, 2).reshape(128, 128).copy(),
                "altp": np.concatenate([al_t, tp_t], axis=1).copy(),
            }
        )
    return in_maps


def _install_ntff_hook():
    """Provide antenv.axon_hooks (missing in this image) so that
    run_bass_kernel_spmd(trace=True) can capture NTFF profiles via the
    axon PJRT .so (same mechanism as trn_agent_boot.trn_boot)."""
    import contextlib
    import ctypes
    import types

    try:
        from antenv.axon_hooks import get_axon_ntff_profile_hook  # noqa: F401

        return True
    except ImportError:
        pass
    so_path = "/opt/axon/libaxon_pjrt.so"
    if not os.path.exists(so_path):
        return False
    lib = ctypes.CDLL(so_path)
    if not hasattr(lib, "axon_start_nrt_profile"):
        return False
    lib.axon_start_nrt_profile.argtypes = [
        ctypes.POINTER(ctypes.c_int64),
        ctypes.c_size_t,
    ]
    lib.axon_start_nrt_profile.restype = ctypes.c_int64
    lib.axon_stop_nrt_profile.argtypes = [ctypes.c_char_p]
    lib.axon_stop_nrt_profile.restype = ctypes.c_int64

    @contextlib.contextmanager
    def _hook(output_dir, device_ids):
        import jax

        jax.devices()
        if device_ids:
            ids = (ctypes.c_int64 * len(device_ids))(*device_ids)
            rc = lib.axon_start_nrt_profile(ids, len(device_ids))
        else:
            rc = lib.axon_start_nrt_profile(None, 0)
        if rc != 0:
            raise RuntimeError(f"axon_start_nrt_profile rc={rc}")
        try:
            yield
        finally:
            n = lib.axon_stop_nrt_profile(str(output_dir).encode())
            print(f"ntff profile: {n} file(s) written to {output_dir}")

    import antenv

    mod = types.ModuleType("antenv.axon_hooks")
    mod.get_axon_ntff_profile_hook = lambda: _hook
    mod.set_axon_ntff_profile_hook = lambda h: None
    sys.modules["antenv.axon_hooks"] = mod
    antenv.axon_hooks = mod
    return True


def kernel(alpha_weights, perm_vectors, temperature):
    global LAST_EXEC_NS, LAST_RESULTS
    from concourse.bass_utils import run_bass_kernel_spmd

    if "nc" not in _cached:
        _cached["nc"] = _build_bass()
    nc = _cached["nc"]
    in_maps = _prep_inputs(alpha_weights, perm_vectors, temperature)
    core_ids = list(range(NCORES))
    trace = os.environ.get("KERNEL_TRACE", "0") == "1"
    if trace:
        trace = _install_ntff_hook()
    try:
        res = run_bass_kernel_spmd(nc, in_maps, core_ids, trace=trace)
    except Exception:
        if not trace:
            raise
        res = run_bass_kernel_spmd(nc, in_maps, core_ids, trace=False)
    LAST_EXEC_NS = res.exec_time_ns
    LAST_RESULTS = res
    out = np.concatenate([res.results[c]["out"] for c in range(NCORES)], axis=0)
    return out.astype(np.float32)


if __name__ == "__main__":
    rng = np.random.default_rng(0)
    a = rng.standard_normal(K).astype(np.float32)
    perm = np.stack([rng.permutation(N) for _ in range(K)]).astype(np.int64)
    T = np.ones((), np.float32)
    out = kernel(a, perm, T)
    # numpy reference
    al = np.exp(a / T - (a / T).max())
    al /= al.sum()
    exp = np.zeros((N, N), np.float32)
    np.add.at(exp, (np.broadcast_to(np.arange(N), (K, N)), perm), al[:, None])
    print("max abs err:", np.abs(out - exp).max(), "max ref:", np.abs(exp).max())
    print("exec ns:", LAST_EXEC_NS)


# revision 105
# speedup vs baseline: 1.2892x; 1.0242x over previous
"""AlphaPermutationLayer Trainium2 kernel.

out[i, j] = sum_k softmax(alpha/T)[k] * (perm[k, i] == j),  N=2048, K=64.

Sharding: output ROWS across the 8 cores (row i depends only on perm[:, i]
and alpha — no collective).  Per core (256 rows), digit-split
j = jq*64 + jf (jq in [0,32), jf in [0,64)); one matmul per row i:
    out_i[jq, jf] = sum_k A_i[k, jq] * B_i[k, jf]
with A = alpha-scaled onehot(perm>>6) stationary ([64, 32]) and
B = onehot(perm&63) moving ([64, 64]); partition p = k + 64*h holds row
half h so DVE builds use all 128 lanes while each matmul contracts 64
partitions at tile_position (64h, 32g).  Single bf16 pass (no hi/lo):
alpha rounding ~2e-3 rel, gate is 2e-2.

Key layout trick: one-hots are stored [p, i2_hi, digit, i2_lo] with the
LOW two i2 bits innermost (IL=4).  The is_equal in0 is then just ph/pl
viewed [p, i2h, 1->digit, i2l] — real data with stride-1 inner, so the
DVE runs in 2x mode with no host-side expansion — and the matmul reads
digit columns at stride 8B (2 per 16B SBUF line).  (Fully-strided PE
reads touch a new SBUF cacheline every element and throttle concurrent
DVE/ACT ops 2-3x; fully-contiguous reads need a 2MB host-expanded
input.  IL=4 gets both cheaply.)  Choosing the col-group g = i2_lo makes
psum partition (32g+jq) merge with the DRAM (row, jq) dims so each bank
drains in ONE 3-dim dma_start.  Matmuls stay in a single serialized PE
stream (h outer): concurrent col-group streams measure slower end-to-end
due to SBUF read pressure.  PSUM holds the whole 2MB per-core output;
ACT evacuates with the fused 1/S softmax normalization; all drain DMAs
issue from the sync HWDGE ring while evacs run on ACT.
"""

import os
import sys

sys.path.insert(0, "/opt/trn_rl_repo")

import numpy as np

N = 2048
K = 64
NCORES = 8
ROWS = N // NCORES          # 256 rows per core
Q = 32                      # stationary digit width (jq)
F = 64                      # moving digit width (jf)
CW = 32                     # i2 chunk width (4 chunks of 32)
IL = 4                      # low i2 bits kept innermost in one-hot layout

LAST_EXEC_NS = None
LAST_RESULTS = None

_cached = {}


def _build_bass():
    import concourse.tile as tile
    from concourse import bacc, mybir

    fp32 = mybir.dt.float32
    bf16 = mybir.dt.bfloat16
    i16 = mybir.dt.int16
    Copy = mybir.ActivationFunctionType.Copy
    Exp = mybir.ActivationFunctionType.Exp
    IsEq = mybir.AluOpType.is_equal

    nc = bacc.Bacc()

    ph_ext = nc.declare_dram_parameter("ph", [128, 128], i16, isOutput=False)
    pl_ext = nc.declare_dram_parameter("pl", [128, 128], i16, isOutput=False)
    at_ext = nc.declare_dram_parameter("altp", [128, 2], fp32, isOutput=False)
    out_ext = nc.declare_dram_parameter("out", [ROWS, N], fp32, isOutput=True)

    with tile.TileContext(nc) as tc:
        with (
            tc.tile_pool(name="sbuf", bufs=1) as sb,
            tc.tile_pool(name="stage", bufs=10) as stp,
            tc.tile_pool(name="smax_psum", bufs=1, space="PSUM") as psmax,
            tc.tile_pool(name="psum", bufs=7, space="PSUM") as pp,
        ):
            # ---- input loads: 2 DMAs; iotas generated on-chip ---------------
            # One-hot tensors are laid out [p, i2h, digit, i2l] with the LOW
            # i2 bits innermost (IL=4): the is_equal in0 is just ph/pl viewed
            # [p, i2h, 1->digit, i2l] — real data, stride-1 inner, 2x DVE
            # mode with NO host expansion.  The matmul then reads digit cols
            # at stride IL*2 = 8B (2 per 16B SBUF line), cutting the line
            # thrash that throttles concurrent DVE/ACT ops.
            ph_t = sb.tile([128, 128], i16)
            pl_t = sb.tile([128, 128], i16)
            at_t = sb.tile([128, 2], fp32)
            nc.sync.dma_start(out=ph_t[:, 0:64], in_=ph_ext[:, 0:64])
            nc.scalar.dma_start(out=ph_t[:, 64:128], in_=ph_ext[:, 64:128])
            nc.sync.dma_start(out=pl_t[:], in_=pl_ext[:])
            nc.scalar.dma_start(out=at_t[:], in_=at_ext[:])
            ph_v = ph_t[:].rearrange("p (ih il) -> p ih il", il=IL)
            pl_v = pl_t[:].rearrange("p (ih il) -> p ih il", il=IL)
            iq_t = sb.tile([128, Q, IL], i16)   # [p, q, il] = q
            if_t = sb.tile([128, F, IL], i16)   # [p, f, il] = f
            nc.gpsimd.iota(iq_t[:], pattern=[[1, Q], [0, IL]], channel_multiplier=0)
            nc.gpsimd.iota(if_t[:], pattern=[[1, F], [0, IL]], channel_multiplier=0)
            al_t = at_t[:, 0:1]
            tp_t = at_t[:, 1:2]

            # ---- softmax head ----------------------------------------------
            # e = exp(alpha/T) unnormalized; S recovered via matmul with 0.5
            # (partitions hold k twice), 1/S applied at evacuation.
            rt_t = sb.tile([128, 1], fp32)
            e_t = sb.tile([128, 1], fp32)
            ln2_t = sb.tile([128, 1], fp32)
            prime_t = sb.tile([128, 1], fp32)
            half_col = sb.tile([128, 1], fp32)
            ones_row = sb.tile([1, 128], fp32)
            r_t = sb.tile([1, 1], fp32)
            rs_t = sb.tile([128, 1], fp32)
            scr_t = sb.tile([128, 512], bf16)   # warmup scratch (uninit junk)
            nc.vector.memset(scr_t[:, 0:2], 1.0)
            nc.vector.memset(ln2_t[:], float(np.log(2.0)))
            nc.vector.memset(half_col[:], 0.5)
            nc.vector.memset(ones_row[:], 1.0)
            # dep-free ACT op: hoists the one-time activation-table load off
            # the exp critical path.
            nc.scalar.activation(out=prime_t[:], in_=ln2_t[:], func=Exp)
            warm_ps = psmax.tile([1, 512], fp32, tag="smax")
            sum_ps = psmax.tile([1, 1], fp32, tag="smax")
            # HAM pre-warm: a few WIDE dep-free matmuls (~3.8us of PE busy —
            # a full 3.4us HAM window) so the clock gate reaches 8/8 before
            # the real stream, without clogging the PE queue ahead of the
            # softmax sum matmul.
            for _ in range(9):
                nc.tensor.matmul(
                    warm_ps[:], lhsT=scr_t[:, 0:1], rhs=scr_t[:],
                    start=True, stop=True,
                )

            IH = 128 // IL                       # i2h extent (32)
            CH = IH // 4                         # i2h per chunk (8)
            d_t = sb.tile([128, IH, Q, IL], i16)   # ph - jq  (0 at the hot q)
            a_s = sb.tile([128, IH, Q, IL], bf16)  # alpha-scaled one-hot
            b_t = sb.tile([128, IH, F, IL], bf16)  # [p, i2h, jf, i2l]

            # DRAM view: row i = 32b + 4s + g, col j = q*64 + f;
            # psum partition = 32g + q, psum free = 64s + f.  With g the LOW
            # row bits, the DRAM dims (g, q) merge to one stride-64 dim, so
            # the whole bank drains in ONE 3-dim dma_start.
            oview = out_ext[:].rearrange(
                "(b s g) (q f) -> b g q s f", b=8, s=8, g=4, q=Q, f=F
            )

            banks = [None] * 8
            rb_ps = []

            def emit_builds(c, lo, ln):
                ih = slice(CH * c + lo, CH * c + lo + ln)
                # subtract instead of is_equal (same 2x cost), so the scale
                # pass can FUSE compare+scale: a_s = (d==0)*e in one 4x
                # tensor_scalar — drops the separate one-hot intermediate.
                nc.vector.tensor_tensor(
                    out=d_t[:, ih],
                    in0=ph_v[:, ih].unsqueeze(2).to_broadcast([128, ln, Q, IL]),
                    in1=iq_t[:].unsqueeze(1).to_broadcast([128, ln, Q, IL]),
                    op=mybir.AluOpType.subtract,
                )
                nc.vector.tensor_tensor(
                    out=b_t[:, ih],
                    in0=pl_v[:, ih].unsqueeze(2).to_broadcast([128, ln, F, IL]),
                    in1=if_t[:].unsqueeze(1).to_broadcast([128, ln, F, IL]),
                    op=IsEq,
                )
                return ih

            def emit_chunk(c):
                # (splitting the last chunk's builds to overlap banks 3/7's
                # first matmuls measured ~1.8us SLOWER — scheduler/sem churn)
                split3 = False
                ih = emit_builds(c, 0, 4 if split3 else CH)
                if c == 0:
                    # alpha chain after the builds (altp can land after ph;
                    # this keeps the DVE bubble-free) — exp runs on ACT in
                    # parallel so e_t is ready before a_s0 issues.
                    nc.vector.reciprocal(out=rt_t[:], in_=tp_t[:])
                    nc.scalar.activation(
                        out=e_t[:], in_=al_t[:], func=Exp, scale=rt_t[:]
                    )
                    nc.tensor.matmul(
                        sum_ps[:], lhsT=e_t[:], rhs=half_col[:],
                        start=True, stop=True,
                    )
                nc.vector.tensor_scalar(
                    out=a_s[:, ih], in0=d_t[:, ih], scalar1=0.0,
                    scalar2=e_t[:], op0=IsEq, op1=mybir.AluOpType.mult,
                )
                if c == 0:
                    nc.vector.reciprocal(out=r_t[:], in_=sum_ps[:])
                # Per h-half: 32 matmuls (h outer keeps the PE a single
                # serialized stream; g inner overlaps LDW), then that bank's
                # drain IMMEDIATELY — the evac's wait-sem then covers only
                # this bank's matmuls, so every bank drains ~1.2us earlier
                # than with a chunk-trailing drain.  Drain DMAs issue from
                # the sync ring (idle), evacs run on ACT.
                def emit_mms(srange):
                    for h in range(2):
                        kp = slice(64 * h, 64 * h + 64)
                        for s in srange:
                            for g in range(4):
                                # row r = 128h + i2, i2 = 4*(CH*c+s) + g
                                nc.tensor.matmul(
                                    banks[c + 4 * h][32 * g : 32 * g + 32, s],
                                    lhsT=a_s[kp, CH * c + s, :, g],
                                    rhs=b_t[kp, CH * c + s, :, g],
                                    start=True,
                                    stop=True,
                                    tile_position=(64 * h, 32 * g),
                                )

                for h in range(2):
                    banks[c + 4 * h] = pp.tile(
                        [128, 8, F], fp32, tag="bank", name=f"bank{c}_{h}"
                    )
                if split3:
                    emit_mms(range(4))
                    emit_builds(c, 4, 4)
                    nc.vector.tensor_scalar(
                        out=a_s[:, CH * c + 4 : CH * c + 8],
                        in0=a_t[:, CH * c + 4 : CH * c + 8], scalar1=e_t[:],
                        scalar2=None, op0=mybir.AluOpType.mult,
                    )
                    emit_mms(range(4, 8))
                    for h in range(2):
                        bi = c + 4 * h
                        stage = stp.tile(
                            [128, 8, F], fp32, tag="stage", name="stage"
                        )
                        nc.scalar.activation(
                            out=stage[:], in_=banks[bi][:], func=Copy,
                            scale=rs_t[:],
                        )
                        nc.sync.dma_start(out=oview[bi], in_=stage[:])
                    return
                for h in range(2):
                    bi = c + 4 * h
                    kp = slice(64 * h, 64 * h + 64)
                    for s in range(8):
                        for g in range(4):
                            # row r = 128h + i2, i2 = 4*(CH*c+s) + g
                            nc.tensor.matmul(
                                banks[bi][32 * g : 32 * g + 32, s],
                                lhsT=a_s[kp, CH * c + s, :, g],
                                rhs=b_t[kp, CH * c + s, :, g],
                                start=True,
                                stop=True,
                                tile_position=(64 * h, 32 * g),
                            )
                    if c == 0 and h == 0:
                        # softmax-tail broadcast BETWEEN bank 0's matmuls
                        # and its evac: placing it before the matmuls made
                        # them wait ~0.55us on r_t for nothing.
                        rb = psmax.tile(
                            [128, 1], fp32, tag="smax", name="rb_ps"
                        )
                        rb_ps.append(rb)
                        nc.tensor.matmul(
                            rb[:], lhsT=ones_row[:], rhs=r_t[:],
                            start=True, stop=True,
                        )
                        nc.vector.tensor_copy(out=rs_t[:], in_=rb[:])
                    stage = stp.tile(
                        [128, 8, F], fp32, tag="stage", name="stage"
                    )
                    nc.scalar.activation(
                        out=stage[:], in_=banks[bi][:], func=Copy,
                        scale=rs_t[:],
                    )
                    nc.sync.dma_start(out=oview[bi], in_=stage[:])

            for c in range(4):
                emit_chunk(c)

    if not nc.is_finalized():
        nc.finalize()
    return nc


def _prep_inputs(alpha_weights, perm_vectors, temperature):
    a = np.asarray(alpha_weights, dtype=np.float32).reshape(K)
    T = np.asarray(temperature, dtype=np.float32).reshape(())
    perm = np.asarray(perm_vectors).astype(np.int64).reshape(K, N)
    ph = (perm >> 6).astype(np.int16)
    pl = (perm & 63).astype(np.int16)
    al_t = np.concatenate([a, a])[:, None].copy()          # [128, 1]
    tp_t = np.full((128, 1), T, dtype=np.float32)
    in_maps = []
    for c in range(NCORES):
        # partition p = k + 64*h, column i2: row r = 128*h + i2 of this core
        phc = ph[:, c * ROWS : (c + 1) * ROWS].reshape(K, 2, 128)
        plc = pl[:, c * ROWS : (c + 1) * ROWS].reshape(K, 2, 128)
        in_maps.append(
            {
                "ph": phc.transpose(1, 0, 2).reshape(128, 128).copy(),
                "pl": plc.transpose(1, 0, 2).reshape(128, 128).copy(),
                "altp": np.concatenate([al_t, tp_t], axis=1).copy(),
            }
        )
    return in_maps


def _install_ntff_hook():
    """Provide antenv.axon_hooks (missing in this image) so that
    run_bass_kernel_spmd(trace=True) can capture NTFF profiles via the
    axon PJRT .so (same mechanism as trn_agent_boot.trn_boot)."""
    import contextlib
    import ctypes
    import types

    try:
        from antenv.axon_hooks import get_axon_ntff_profile_hook  # noqa: F401

        return True
    except ImportError:
        pass
    so_path = "/opt/axon/libaxon_pjrt.so"
    if not os.path.exists(so_path):
        return False
    lib = ctypes.CDLL(so_path)
    if not hasattr(lib, "axon_start_nrt_profile"):
        return False
    lib.axon_start_nrt_profile.argtypes = [
        ctypes.POINTER(ctypes.c_int64),
        ctypes.c_size_t,
    ]
    lib.axon_start_nrt_profile.restype = ctypes.c_int64
    lib.axon_stop_nrt_profile.argtypes = [ctypes.c_char_p]
    lib.axon_stop_nrt_profile.restype = ctypes.c_int64

    @contextlib.contextmanager
    def _hook(output_dir, device_ids):
        import jax

        jax.devices()
        if device_ids:
            ids = (ctypes.c_int64 * len(device_ids))(*device_ids)
            rc = lib.axon_start_nrt_profile(ids, len(device_ids))
        else:
            rc = lib.axon_start_nrt_profile(None, 0)
        if rc != 0:
            raise RuntimeError(f"axon_start_nrt_profile rc={rc}")
        try:
            yield
        finally:
            n = lib.axon_stop_nrt_profile(str(output_dir).encode())
            print(f"ntff profile: {n} file(s) written to {output_dir}")

    import antenv

    mod = types.ModuleType("antenv.axon_hooks")
    mod.get_axon_ntff_profile_hook = lambda: _hook
    mod.set_axon_ntff_profile_hook = lambda h: None
    sys.modules["antenv.axon_hooks"] = mod
    antenv.axon_hooks = mod
    return True


def kernel(alpha_weights, perm_vectors, temperature):
    global LAST_EXEC_NS, LAST_RESULTS
    from concourse.bass_utils import run_bass_kernel_spmd

    if "nc" not in _cached:
        _cached["nc"] = _build_bass()
    nc = _cached["nc"]
    in_maps = _prep_inputs(alpha_weights, perm_vectors, temperature)
    core_ids = list(range(NCORES))
    trace = os.environ.get("KERNEL_TRACE", "0") == "1"
    if trace:
        trace = _install_ntff_hook()
    try:
        res = run_bass_kernel_spmd(nc, in_maps, core_ids, trace=trace)
    except Exception:
        if not trace:
            raise
        res = run_bass_kernel_spmd(nc, in_maps, core_ids, trace=False)
    LAST_EXEC_NS = res.exec_time_ns
    LAST_RESULTS = res
    out = np.concatenate([res.results[c]["out"] for c in range(NCORES)], axis=0)
    return out.astype(np.float32)


if __name__ == "__main__":
    rng = np.random.default_rng(0)
    a = rng.standard_normal(K).astype(np.float32)
    perm = np.stack([rng.permutation(N) for _ in range(K)]).astype(np.int64)
    T = np.ones((), np.float32)
    out = kernel(a, perm, T)
    # numpy reference
    al = np.exp(a / T - (a / T).max())
    al /= al.sum()
    exp = np.zeros((N, N), np.float32)
    np.add.at(exp, (np.broadcast_to(np.arange(N), (K, N)), perm), al[:, None])
    print("max abs err:", np.abs(out - exp).max(), "max ref:", np.abs(exp).max())
    print("exec ns:", LAST_EXEC_NS)
